# revision 1
# baseline (speedup 1.0000x reference)
"""ChannelMerger kernel for 8x Trainium2 NeuronCores (Bass/Tile).

Computes, for eeg [B,T,C], positions [B,C,2], heads [O,D]:
    emb     = fourier_emb(positions)              # [B,C,D], D = 2*12*12
    scores  = einsum('bcd,od->boc', emb, heads)   # [B,O,C]
    weights = softmax(scores, axis=2)
    out     = einsum('bct,boc->bot', eeg_ct, weights).transpose -> [B,T,O]

Sharding: data-parallel over batch B=32 -> 4 batches per core on 8 cores.
All compute (fourier, matmuls, softmax, weighted sum) runs on-device; the
host only shards/reshapes inputs and pads constants.
"""

import numpy as np

import concourse.bacc as bacc
import concourse.mybir as mybir
import concourse.tile as tile

# ---------------------------------------------------------------- constants
B, T, C = 32, 8192, 128
O = 64
N_FREQS = 12
N_IJ = N_FREQS * N_FREQS          # 144
D = 2 * N_IJ                      # 288
MARGIN = 0.2
N_CORES = 8
BPC = B // N_CORES                # batches per core = 4
TGROUP = 512                      # t rows per group
NGROUP = T // TGROUP              # 16
JI = 4                            # row interleave within a group
F32 = mybir.dt.float32


# ------------------------------------------------------------ host constants
def _host_constants(heads: np.ndarray):
    """Pure layout/padding transforms of `heads` + static tables."""
    width = 1.0 + 2.0 * MARGIN
    # Frequencies in TURNS (cycles): loc_rad = 2*pi * (pos_x*p_i + pos_y*p_j).
    # Working in turns lets the device reduce the phase into [-pi, pi] with a
    # round-to-nearest int cast before the Sin table lookup.
    p = np.arange(N_FREQS, dtype=np.float64) / width

    # Per-partition frequency columns for the transposed loc computation.
    # Chunk c covers ij = 128c + k (k = partition); entries past 143 are 0
    # and their heads rows are zero-padded, so they contribute nothing.
    pij = np.zeros((128, 4), dtype=np.float32)
    for c in range(2):
        for k in range(128):
            ij = 128 * c + k
            if ij < N_IJ:
                pij[k, 2 * c + 0] = p[ij // N_FREQS]
                pij[k, 2 * c + 1] = p[ij % N_FREQS]

    # headsT chunks [K=128, O] for the 4 embT chunks (cos0, cos1, sin0, sin1)
    ht4 = np.zeros((128, 4 * O), dtype=np.float32)
    ht4[:, 0 * O:1 * O] = heads[:, 0:128].T               # cos ij 0..127
    ht4[:16, 1 * O:2 * O] = heads[:, 128:144].T           # cos ij 128..143
    ht4[:, 2 * O:3 * O] = heads[:, 144:272].T             # sin ij 0..127
    ht4[:16, 3 * O:4 * O] = heads[:, 272:288].T           # sin ij 128..143

    ident = np.eye(128, dtype=np.float32)
    return pij, ht4, ident


def _pos_broadcast(positions_core: np.ndarray) -> np.ndarray:
    """[BPC,C,2] -> [128, BPC*256] with (x+MARGIN | y+MARGIN) per batch,
    replicated across all 128 partitions (layout-only transform)."""
    pos = positions_core.astype(np.float32) + np.float32(MARGIN)
    row = np.concatenate(
        [np.concatenate([pos[b, :, 0], pos[b, :, 1]]) for b in range(BPC)]
    )  # [BPC*256]
    return np.broadcast_to(row, (128, row.size)).copy()


# ------------------------------------------------------------- device kernel
def _build_nc(debug=False):
    # Bacc (not plain Bass): finalize() runs generate_event_semaphores,
    # which splits multi-sem waits (TRN2 allows 1 wait per instruction).
    nc = bacc.Bacc()
    eeg = nc.declare_dram_parameter("eeg", [BPC, T, C], F32, isOutput=False)
    posb = nc.declare_dram_parameter("posb", [128, BPC * 2 * C], F32, isOutput=False)
    ht4 = nc.declare_dram_parameter("ht4", [128, 4 * O], F32, isOutput=False)
    pij = nc.declare_dram_parameter("pij", [128, 4], F32, isOutput=False)
    identity = nc.declare_dram_parameter("identity", [128, 128], F32, isOutput=False)
    out = nc.declare_dram_parameter("out", [BPC, T, O], F32, isOutput=True)
    if debug:
        wt_out = nc.declare_dram_parameter("wt_out", [128, BPC * O], F32, isOutput=True)
        emb_out = nc.declare_dram_parameter("emb_out", [128, 4 * 128], F32, isOutput=True)

    TWO_PI = float(2.0 * np.pi)
    I32 = mybir.dt.int32
    BF16 = mybir.dt.bfloat16

    with tile.TileContext(nc) as tc:
        with tc.tile_pool(name="consts", bufs=1) as cpool:
            # PE warm-up: the HAM clock gate keeps the PE at 1.2 GHz until it
            # sees ~3.4us of sustained matmul activity. Burn a burst of cheap
            # bf16 matmuls while the initial DMAs land so the real work runs
            # at 2.4 GHz from the start.
            wu_a = cpool.tile([128, 128], BF16)
            wu_b = cpool.tile([128, 512], BF16)
            nc.vector.memset(wu_a, 1.0)
            nc.vector.memset(wu_b, 1.0)
            with tc.tile_pool(name="wups", bufs=1, space="PSUM") as wups:
                wu_ps = wups.tile([128, 512], F32)
                for _ in range(24):
                    nc.tensor.matmul(out=wu_ps, lhsT=wu_a, rhs=wu_b,
                                     start=True, stop=True)
            posb_sb = cpool.tile([128, BPC * 2 * C], F32)
            nc.sync.dma_start(out=posb_sb, in_=posb[:, :])
            pij_sb = cpool.tile([128, 4], F32)
            nc.sync.dma_start(out=pij_sb, in_=pij[:, :])
            ht4_sb = cpool.tile([128, 4 * O], F32)
            nc.sync.dma_start(out=ht4_sb, in_=ht4[:, :])
            ident_sb = cpool.tile([128, 128], F32)
            nc.sync.dma_start(out=ident_sb, in_=identity[:, :])
            # softmaxed channel weights, transposed: [C, O] per batch
            wt_all = cpool.tile([128, BPC * O], F32)

            with (
                tc.tile_pool(name="ein", bufs=2) as ein,
                tc.tile_pool(name="wsb", bufs=1) as wsb,
                tc.tile_pool(name="ets", bufs=16) as ets,
                tc.tile_pool(name="osb", bufs=2) as osb,
                tc.tile_pool(name="wps", bufs=1, space="PSUM") as wps,
                tc.tile_pool(name="etp", bufs=4, space="PSUM") as etp,
                tc.tile_pool(name="otp", bufs=2, space="PSUM") as otp,
            ):

                # Kick off all eeg loads first: one bulk 4 MB DMA per batch,
                # double-buffered (b>=2 waits for slot release), overlapping
                # the weights computation below.
                e_tiles = []
                for b in range(BPC):
                    e_sb = ein.tile([128, NGROUP * TGROUP], F32, tag="e", name=f"e_{b}")
                    eeg_r = eeg[b].rearrange("(g p j) c -> p g (j c)", p=128, j=JI)
                    nc.sync.dma_start(
                        out=e_sb.rearrange("p (g x) -> p g x", g=NGROUP), in_=eeg_r
                    )
                    e_tiles.append(e_sb)

                # ---------- phase 0: fourier emb + scores + softmax --------
                # All 4 batches processed in single wide ops where possible.
                pv = posb_sb.rearrange("p (b s c) -> p b s c", b=BPC, s=2)
                x_all = pv[:, :, 0, :]   # [128, BPC, C]
                y_all = pv[:, :, 1, :]
                embq = wsb.tile([128, BPC, 4, 128], F32, tag="embq")
                for c in range(2):
                    # phase in turns: t = x*p_i + y*p_j  (>= 0, < ~19)
                    t1 = wsb.tile([128, BPC, 128], F32, tag="t1")
                    tt = wsb.tile([128, BPC, 128], F32, tag="tt")
                    nc.vector.tensor_scalar_mul(
                        out=t1, in0=x_all, scalar1=pij_sb[:, 2 * c:2 * c + 1]
                    )
                    nc.vector.tensor_scalar_mul(
                        out=tt, in0=y_all, scalar1=pij_sb[:, 2 * c + 1:2 * c + 2]
                    )
                    nc.vector.tensor_add(out=tt, in0=tt, in1=t1)
                    tc4 = wsb.tile([128, BPC, 128], F32, tag="tc4")
                    nc.vector.tensor_scalar_add(out=tc4, in0=tt, scalar1=0.25)
                    # cos chunk (t+0.25) -> q=c, sin chunk -> q=2+c.
                    # Reduce phase via round-to-nearest-even f32->i32 cast:
                    # r = t - rne(t) in [-0.5, 0.5]; sin(2pi*t) = Sin(2pi*r).
                    for src_t, q in ((tc4, c), (tt, 2 + c)):
                        ki = wsb.tile([128, BPC, 128], I32, tag="ki")
                        kf = wsb.tile([128, BPC, 128], F32, tag="kf")
                        nc.vector.tensor_copy(out=ki, in_=src_t)
                        nc.vector.tensor_copy(out=kf, in_=ki)
                        rr = wsb.tile([128, BPC, 128], F32, tag="rr")
                        nc.vector.tensor_sub(out=rr, in0=src_t, in1=kf)
                        nc.scalar.activation(
                            out=embq[:, :, q, :], in_=rr,
                            func=mybir.ActivationFunctionType.Sin,
                            scale=TWO_PI, bias=0.0,
                        )
                if debug:
                    nc.sync.dma_start(
                        out=emb_out[:, :],
                        in_=embq[:, 0, :, :],
                    )
                scores_ps = wps.tile([O, BPC, 128], F32, tag="scores")
                for b in range(BPC):
                    for q in range(4):
                        nc.tensor.matmul(
                            out=scores_ps[:, b, :],
                            lhsT=ht4_sb[:, q * O:(q + 1) * O],
                            rhs=embq[:, b, q, :],
                            start=(q == 0), stop=(q == 3),
                        )
                # scores are bounded (|s| < ~10): plain exp is fp32-safe and
                # softmax is shift-invariant, so skip the max-subtraction —
                # one less DVE hop on the critical path to the weights.
                probs = wsb.tile([O, BPC, 128], F32, tag="probs")
                ssum = wsb.tile([O, BPC], F32, tag="ssum")
                for b in range(BPC):
                    nc.scalar.activation(
                        out=probs[:, b, :], in_=scores_ps[:, b, :],
                        func=mybir.ActivationFunctionType.Exp,
                        bias=0.0, accum_out=ssum[:, b:b + 1],
                    )
                # Prologue: batch-0's first two groups of eeg transposes
                # (PSUM-bank-limited to 2 groups) keep the PE busy while the
                # softmax chain below resolves, instead of stalling the
                # in-order PE queue at the weight transposes.
                NPRO = 2
                pro_ets = {}
                for g in range(NPRO):
                    eg = e_tiles[0][:, g * TGROUP:(g + 1) * TGROUP]
                    pair = []
                    for h in range(2):
                        et_ps = etp.tile([128, 256], F32, tag="etps",
                                         name=f"pro_etps_{g}_{h}")
                        for jj in range(2):
                            j = 2 * h + jj
                            nc.tensor.transpose(
                                out=et_ps[:, jj * 128:(jj + 1) * 128],
                                in_=eg[:, j * 128:(j + 1) * 128],
                                identity=ident_sb,
                            )
                        et_sb = ets.tile([128, 256], F32, tag="etsb",
                                         name=f"pro_etsb_{g}_{h}")
                        nc.vector.tensor_copy(out=et_sb, in_=et_ps)
                        pair.append(et_sb)
                    pro_ets[g] = pair

                rcp = wsb.tile([O, BPC], F32, tag="rcp")
                nc.vector.reciprocal(out=rcp, in_=ssum)
                wgt = wsb.tile([O, BPC, 128], F32, tag="wgt")
                wt_ps = wps.tile([128, BPC, O], F32, tag="wtps")
                for b in range(BPC):
                    nc.vector.tensor_scalar_mul(
                        out=wgt[:, b, :], in0=probs[:, b, :],
                        scalar1=rcp[:, b:b + 1],
                    )
                    nc.tensor.transpose(
                        out=wt_ps[:, b, :], in_=wgt[:, b, :],
                        identity=ident_sb[0:O, 0:O],
                    )
                nc.vector.tensor_copy(out=wt_all, in_=wt_ps)
                if debug:
                    nc.sync.dma_start(out=wt_out[:, :], in_=wt_all)

                # ---------- main loop: out[t,o] = sum_c eeg[t,c]*w[o,c] ----
                for b in range(BPC):
                    out_r = out[b].rearrange("(g p j) o -> p g (j o)", p=128, j=JI)
                    wt_b = wt_all[:, b * O:(b + 1) * O]
                    e_sb = e_tiles[b]
                    o_sb = osb.tile([128, NGROUP * JI * O], F32, tag="osb")
                    for g in range(NGROUP):
                        eg = e_sb[:, g * TGROUP:(g + 1) * TGROUP]
                        out_ps = otp.tile([128, JI * O], F32, tag="outps")
                        for h in range(2):  # transpose pairs
                            if b == 0 and g < NPRO:
                                et_sb = pro_ets[g][h]  # transposed in prologue
                            else:
                                et_ps = etp.tile([128, 256], F32, tag="etps")
                                for jj in range(2):
                                    j = 2 * h + jj
                                    nc.tensor.transpose(
                                        out=et_ps[:, jj * 128:(jj + 1) * 128],
                                        in_=eg[:, j * 128:(j + 1) * 128],
                                        identity=ident_sb,
                                    )
                                et_sb = ets.tile([128, 256], F32, tag="etsb")
                                nc.vector.tensor_copy(out=et_sb, in_=et_ps)
                            for jj in range(2):
                                j = 2 * h + jj
                                nc.tensor.matmul(
                                    out=out_ps[:, j * O:(j + 1) * O],
                                    lhsT=et_sb[:, jj * 128:(jj + 1) * 128],
                                    rhs=wt_b,
                                    start=True, stop=True,
                                )
                        nc.scalar.copy(
                            out=o_sb[:, g * JI * O:(g + 1) * JI * O], in_=out_ps
                        )
                        nsp = 4 if b == BPC - 1 else 2  # finer drain at the tail
                        gper = NGROUP // nsp
                        if g % gper == gper - 1:
                            part = g // gper
                            hw = gper * JI * O
                            nc.sync.dma_start(
                                out=out_r[:, part * gper:(part + 1) * gper, :],
                                in_=o_sb[:, part * hw:(part + 1) * hw].rearrange(
                                    "p (g x) -> p g x", g=gper
                                ),
                            )
    nc.finalize()
    return nc


_NC_CACHE = None


def _get_nc():
    global _NC_CACHE
    if _NC_CACHE is None:
        _NC_CACHE = _build_nc()
    return _NC_CACHE


def _make_in_maps(eeg, positions, heads):
    pij, ht4, ident = _host_constants(np.asarray(heads, dtype=np.float32))
    eeg = np.asarray(eeg, dtype=np.float32)
    positions = np.asarray(positions, dtype=np.float32)
    in_maps = []
    for core in range(N_CORES):
        sl = slice(core * BPC, (core + 1) * BPC)
        in_maps.append({
            "eeg": np.ascontiguousarray(eeg[sl]),
            "posb": _pos_broadcast(positions[sl]),
            "ht4": ht4,
            "pij": pij,
            "identity": ident,
        })
    return in_maps


def kernel(eeg, positions, heads, sub=None, **_unused):
    from concourse.bass_utils import run_bass_kernel_spmd

    nc = _get_nc()
    in_maps = _make_in_maps(eeg, positions, heads)
    res = run_bass_kernel_spmd(nc, in_maps, list(range(N_CORES)))
    out = np.concatenate([res.results[c]["out"] for c in range(N_CORES)], axis=0)
    return out



# revision 2
# speedup vs baseline: 2.2076x; 2.2076x over previous
"""ChannelMerger kernel for 8x Trainium2 NeuronCores (Bass/Tile).

Computes, for eeg [B,T,C], positions [B,C,2], heads [O,D]:
    emb     = fourier_emb(positions)              # [B,C,D], D = 2*12*12
    scores  = einsum('bcd,od->boc', emb, heads)   # [B,O,C]
    weights = softmax(scores, axis=2)
    out     = einsum('bct,boc->bot', eeg_ct, weights).transpose -> [B,T,O]

Sharding: data-parallel over batch B=32 -> 4 batches per core on 8 cores.

Device-side strategy (memory-bound problem):
  - Host pre-transposes eeg to [B, C, T] and casts to bf16 (layout/dtype
    prep only, like the heads/positions packing). This halves the input
    DMA bytes and removes all on-device eeg transposes: the C-contraction
    matmul needs C on partitions, which the transposed layout gives for
    free with perfectly contiguous 16KB DMA runs.
  - The per-batch softmax weight matrix wt [C=128, O=64] is the matmul's
    *stationary* operand (loaded once per 512-col chunk); eegT streams
    through as the moving tensor at 1 col/cycle in bf16. Output appears
    as outT [O, T] in PSUM (fp32), is cast to bf16 on the DVE/Act
    engines, and DMA'd out as [B, O, T]; the host casts/transposes back.
  - Two batches share each PSUM tile ([128, 512] = batch pair on
    partition halves) so the PSUM->SBUF cast copies run full-width.
"""

import numpy as np
import ml_dtypes

import concourse.bacc as bacc
import concourse.mybir as mybir
import concourse.tile as tile

# ---------------------------------------------------------------- constants
B, T, C = 32, 8192, 128
O = 64
N_FREQS = 12
N_IJ = N_FREQS * N_FREQS          # 144
D = 2 * N_IJ                      # 288
MARGIN = 0.2
N_CORES = 8
BPC = B // N_CORES                # batches per core = 4
TGROUP = 512                      # moving-tensor cols per matmul (max 512)
NGROUP = T // TGROUP              # 16
F32 = mybir.dt.float32
BF16 = mybir.dt.bfloat16
BF16_NP = ml_dtypes.bfloat16


# ------------------------------------------------------------ host constants
def _host_constants(heads: np.ndarray):
    """Pure layout/padding transforms of `heads` + static tables."""
    width = 1.0 + 2.0 * MARGIN
    # Frequencies in TURNS (cycles): loc_rad = 2*pi * (pos_x*p_i + pos_y*p_j).
    # Working in turns lets the device reduce the phase into [-pi, pi] with a
    # round-to-nearest int cast before the Sin table lookup.
    p = np.arange(N_FREQS, dtype=np.float64) / width

    # Per-partition frequency columns for the transposed loc computation.
    # Chunk c covers ij = 128c + k (k = partition); entries past 143 are 0
    # and their heads rows are zero-padded, so they contribute nothing.
    pij = np.zeros((128, 4), dtype=np.float32)
    for c in range(2):
        for k in range(128):
            ij = 128 * c + k
            if ij < N_IJ:
                pij[k, 2 * c + 0] = p[ij // N_FREQS]
                pij[k, 2 * c + 1] = p[ij % N_FREQS]

    # headsT chunks [K=128, O] for the 4 embT chunks (cos0, cos1, sin0, sin1)
    ht4 = np.zeros((128, 4 * O), dtype=np.float32)
    ht4[:, 0 * O:1 * O] = heads[:, 0:128].T               # cos ij 0..127
    ht4[:16, 1 * O:2 * O] = heads[:, 128:144].T           # cos ij 128..143
    ht4[:, 2 * O:3 * O] = heads[:, 144:272].T             # sin ij 0..127
    ht4[:16, 3 * O:4 * O] = heads[:, 272:288].T           # sin ij 128..143

    ident = np.eye(128, dtype=np.float32)
    return pij, ht4, ident


def _pos_broadcast(positions_core: np.ndarray) -> np.ndarray:
    """[BPC,C,2] -> [128, BPC*256] with (x+MARGIN | y+MARGIN) per batch,
    replicated across all 128 partitions (layout-only transform)."""
    pos = positions_core.astype(np.float32) + np.float32(MARGIN)
    row = np.concatenate(
        [np.concatenate([pos[b, :, 0], pos[b, :, 1]]) for b in range(BPC)]
    )  # [BPC*256]
    return np.broadcast_to(row, (128, row.size)).copy()


# ------------------------------------------------------------- device kernel
def _build_nc():
    # Bacc (not plain Bass): finalize() runs generate_event_semaphores,
    # which splits multi-sem waits (TRN2 allows 1 wait per instruction).
    nc = bacc.Bacc()
    eegT = nc.declare_dram_parameter("eegT", [BPC, C, T], BF16, isOutput=False)
    posb = nc.declare_dram_parameter("posb", [128, BPC * 2 * C], F32, isOutput=False)
    ht4 = nc.declare_dram_parameter("ht4", [128, 4 * O], F32, isOutput=False)
    pij = nc.declare_dram_parameter("pij", [128, 4], F32, isOutput=False)
    identity = nc.declare_dram_parameter("identity", [128, 128], F32, isOutput=False)
    outT = nc.declare_dram_parameter("outT", [BPC, O, T], BF16, isOutput=True)

    TWO_PI = float(2.0 * np.pi)
    I32 = mybir.dt.int32

    with tile.TileContext(nc) as tc:
        with tc.tile_pool(name="consts", bufs=1) as cpool:
            # PE warm-up: the HAM clock gate keeps the PE at 1.2 GHz until it
            # sees ~3.4us of sustained matmul activity. Burn a burst of cheap
            # bf16 matmuls while the initial DMAs land so the real work runs
            # at 2.4 GHz from the start.
            wu_a = cpool.tile([128, 128], BF16)
            wu_b = cpool.tile([128, 512], BF16)
            nc.vector.memset(wu_a, 1.0)
            nc.vector.memset(wu_b, 1.0)
            with tc.tile_pool(name="wups", bufs=1, space="PSUM") as wups:
                wu_ps = wups.tile([128, 512], F32)
                for _ in range(24):
                    nc.tensor.matmul(out=wu_ps, lhsT=wu_a, rhs=wu_b,
                                     start=True, stop=True)
            posb_sb = cpool.tile([128, BPC * 2 * C], F32)
            nc.sync.dma_start(out=posb_sb, in_=posb[:, :])
            pij_sb = cpool.tile([128, 4], F32)
            nc.sync.dma_start(out=pij_sb, in_=pij[:, :])
            ht4_sb = cpool.tile([128, 4 * O], F32)
            nc.sync.dma_start(out=ht4_sb, in_=ht4[:, :])
            ident_sb = cpool.tile([128, 128], F32)
            nc.sync.dma_start(out=ident_sb, in_=identity[:, :])
            # softmaxed channel weights, transposed: [C, O] per batch (bf16)
            wt_bf = cpool.tile([128, BPC * O], BF16)

            with (
                tc.tile_pool(name="ein", bufs=BPC) as ein,
                tc.tile_pool(name="wsb", bufs=1) as wsb,
                tc.tile_pool(name="osb", bufs=2) as osb,
                tc.tile_pool(name="wps", bufs=1, space="PSUM") as wps,
                tc.tile_pool(name="otp", bufs=4, space="PSUM") as otp,
            ):

                # Kick off all eegT loads first: one bulk 2 MB DMA per batch
                # with 16 KB contiguous runs per partition, overlapping the
                # weights computation below.
                e_tiles = []
                for b in range(BPC):
                    e_sb = ein.tile([128, T], BF16, tag="e", name=f"e_{b}")
                    nc.sync.dma_start(out=e_sb, in_=eegT[b])
                    e_tiles.append(e_sb)

                # ---------- phase 0: fourier emb + scores + softmax --------
                # All 4 batches processed in single wide ops where possible.
                pv = posb_sb.rearrange("p (b s c) -> p b s c", b=BPC, s=2)
                x_all = pv[:, :, 0, :]   # [128, BPC, C]
                y_all = pv[:, :, 1, :]
                embq = wsb.tile([128, BPC, 4, 128], F32, tag="embq")
                for c in range(2):
                    # phase in turns: t = x*p_i + y*p_j  (>= 0, < ~19)
                    t1 = wsb.tile([128, BPC, 128], F32, tag="t1")
                    tt = wsb.tile([128, BPC, 128], F32, tag="tt")
                    nc.vector.tensor_scalar_mul(
                        out=t1, in0=x_all, scalar1=pij_sb[:, 2 * c:2 * c + 1]
                    )
                    nc.vector.tensor_scalar_mul(
                        out=tt, in0=y_all, scalar1=pij_sb[:, 2 * c + 1:2 * c + 2]
                    )
                    nc.vector.tensor_add(out=tt, in0=tt, in1=t1)
                    tc4 = wsb.tile([128, BPC, 128], F32, tag="tc4")
                    nc.vector.tensor_scalar_add(out=tc4, in0=tt, scalar1=0.25)
                    # cos chunk (t+0.25) -> q=c, sin chunk -> q=2+c.
                    # Reduce phase via round-to-nearest-even f32->i32 cast:
                    # r = t - rne(t) in [-0.5, 0.5]; sin(2pi*t) = Sin(2pi*r).
                    for src_t, q in ((tc4, c), (tt, 2 + c)):
                        ki = wsb.tile([128, BPC, 128], I32, tag="ki")
                        kf = wsb.tile([128, BPC, 128], F32, tag="kf")
                        nc.vector.tensor_copy(out=ki, in_=src_t)
                        nc.vector.tensor_copy(out=kf, in_=ki)
                        rr = wsb.tile([128, BPC, 128], F32, tag="rr")
                        nc.vector.tensor_sub(out=rr, in0=src_t, in1=kf)
                        nc.scalar.activation(
                            out=embq[:, :, q, :], in_=rr,
                            func=mybir.ActivationFunctionType.Sin,
                            scale=TWO_PI, bias=0.0,
                        )
                scores_ps = wps.tile([O, BPC, 128], F32, tag="scores")
                for b in range(BPC):
                    for q in range(4):
                        nc.tensor.matmul(
                            out=scores_ps[:, b, :],
                            lhsT=ht4_sb[:, q * O:(q + 1) * O],
                            rhs=embq[:, b, q, :],
                            start=(q == 0), stop=(q == 3),
                        )
                # scores are bounded (|s| < ~10): plain exp is fp32-safe and
                # softmax is shift-invariant, so skip the max-subtraction —
                # one less DVE hop on the critical path to the weights.
                probs = wsb.tile([O, BPC, 128], F32, tag="probs")
                ssum = wsb.tile([O, BPC], F32, tag="ssum")
                for b in range(BPC):
                    nc.scalar.activation(
                        out=probs[:, b, :], in_=scores_ps[:, b, :],
                        func=mybir.ActivationFunctionType.Exp,
                        bias=0.0, accum_out=ssum[:, b:b + 1],
                    )
                rcp = wsb.tile([O, BPC], F32, tag="rcp")
                nc.vector.reciprocal(out=rcp, in_=ssum)
                wgt = wsb.tile([O, BPC, 128], F32, tag="wgt")
                wt_ps = wps.tile([128, BPC, O], F32, tag="wtps")
                for b in range(BPC):
                    nc.vector.tensor_scalar_mul(
                        out=wgt[:, b, :], in0=probs[:, b, :],
                        scalar1=rcp[:, b:b + 1],
                    )
                    nc.tensor.transpose(
                        out=wt_ps[:, b, :], in_=wgt[:, b, :],
                        identity=ident_sb[0:O, 0:O],
                    )
                nc.vector.tensor_copy(out=wt_bf, in_=wt_ps)

                # ---------- main loop: outT[o,t] = sum_c w[c,o]*eegT[c,t] --
                # Batch pairs share one PSUM tile on partition halves so the
                # fp32->bf16 drain copies run at full 128-lane width.
                DRAIN = 4            # groups per output DMA chunk
                for pair in range(BPC // 2):
                    b0, b1 = 2 * pair, 2 * pair + 1
                    ot_sb = osb.tile([128, T], BF16, tag="ot")
                    for g in range(NGROUP):
                        sl = slice(g * TGROUP, (g + 1) * TGROUP)
                        ot_ps = otp.tile([128, TGROUP], F32, tag="otps")
                        nc.tensor.matmul(
                            out=ot_ps[0:O, :],
                            lhsT=wt_bf[:, b0 * O:(b0 + 1) * O],
                            rhs=e_tiles[b0][:, sl],
                            start=True, stop=True,
                        )
                        nc.tensor.matmul(
                            out=ot_ps[O:2 * O, :],
                            lhsT=wt_bf[:, b1 * O:(b1 + 1) * O],
                            rhs=e_tiles[b1][:, sl],
                            start=True, stop=True,
                        )
                        # alternate the cast-copy between DVE and Act engines
                        if g % 2 == 0:
                            nc.vector.tensor_copy(out=ot_sb[:, sl], in_=ot_ps)
                        else:
                            nc.scalar.copy(out=ot_sb[:, sl], in_=ot_ps)
                        if g % DRAIN == DRAIN - 1:
                            part = g // DRAIN
                            csl = slice(part * DRAIN * TGROUP,
                                        (part + 1) * DRAIN * TGROUP)
                            nc.sync.dma_start(
                                out=outT[b0][:, csl], in_=ot_sb[0:O, csl]
                            )
                            nc.sync.dma_start(
                                out=outT[b1][:, csl], in_=ot_sb[O:2 * O, csl]
                            )
    nc.finalize()
    return nc


_NC_CACHE = None


def _get_nc():
    global _NC_CACHE
    if _NC_CACHE is None:
        _NC_CACHE = _build_nc()
    return _NC_CACHE


def _make_in_maps(eeg, positions, heads):
    pij, ht4, ident = _host_constants(np.asarray(heads, dtype=np.float32))
    positions = np.asarray(positions, dtype=np.float32)
    # Layout/dtype prep only: cast once (contiguous), then transpose-copy
    # the bf16 array (half the bytes of transposing fp32).
    eeg_bf = np.asarray(eeg, dtype=np.float32).astype(BF16_NP)
    in_maps = []
    for core in range(N_CORES):
        sl = slice(core * BPC, (core + 1) * BPC)
        in_maps.append({
            "eegT": np.ascontiguousarray(eeg_bf[sl].transpose(0, 2, 1)),
            "posb": _pos_broadcast(positions[sl]),
            "ht4": ht4,
            "pij": pij,
            "identity": ident,
        })
    return in_maps


def kernel(eeg, positions, heads, sub=None, **_unused):
    from concourse.bass_utils import run_bass_kernel_spmd

    nc = _get_nc()
    in_maps = _make_in_maps(eeg, positions, heads)
    res = run_bass_kernel_spmd(nc, in_maps, list(range(N_CORES)))
    outT = np.concatenate(
        [np.asarray(res.results[c]["outT"]) for c in range(N_CORES)], axis=0
    )  # [B, O, T] bf16
    return outT.transpose(0, 2, 1).astype(np.float32)
